# revision 1
# baseline (speedup 1.0000x reference)
"""Trainium2 Bass kernel for nn_AttentionLayer (B=16, TQ=TK=H=1024, fp32).

reference:
    scores  = einsum('bqh,bkh->bqk', query, memory_bank)
    probs   = softmax(scores, axis=2)
    context = einsum('bqk,bkh->bqh', probs, memory_bank)
    return (context, scores)

Sharding: batch dim split across 8 NeuronCores (2 batches per core), no
cross-device communication.

Per-core kernel (per batch):
  - load K natural [k, h] into SBUF; build K^T via PE transposes (rounded to
    fp32r in the PSUM->SBUF copy) and K_r (fp32r copy of K) for the second
    matmul.
  - per 128-row q-tile: transpose Q block to Q^T (fp32r), S = (Q^T)^T @ K^T
    accumulated over h in PSUM, softmax row stats on DVE/ACT (exp produces the
    row sum via accum_out), scores DMA'd straight out of PSUM, E^T via PE
    transposes, C = (E^T)^T @ K_r in PSUM, row-scaled by 1/sum into SBUF, DMA
    out.
All matmuls run in fp32r (TF32-like) at full PE rate; operands are rounded to
fp32r by the copies that stage them into SBUF (required by the BIR verifier).
"""

import numpy as np

import concourse.bass as bass
import concourse.mybir as mybir
import concourse.tile as tile
from concourse import bacc
from concourse.masks import make_identity
from concourse.bass_utils import run_bass_kernel_spmd

N_CORES = 8
B, TQ, TK, H = 16, 1024, 1024, 1024
B_PC = B // N_CORES
P = 128

F32 = mybir.dt.float32
F32R = mybir.dt.float32r


def _mm_chunks(width):
    """Split a free-dim width into <=512 chunks (fp32 moving-operand cap)."""
    n = max(1, (width + 511) // 512)
    assert width % n == 0
    return [(i * (width // n), width // n) for i in range(n)]


def build_attention_nc(b_pc=B_PC, tq=TQ, tk=TK, h=H, repeats=1, strip_dma=False):
    """Build (and compile) the per-core Bass program.

    DRAM tensors: query [b_pc, tq, h], memory_bank [b_pc, tk, h] (inputs);
    scores [b_pc, tq, tk], context [b_pc, tq, h] (outputs). All fp32.
    repeats>1 wraps the whole computation in a hardware loop (timing only).
    """
    nq, nk, nh = tq // P, tk // P, h // P
    assert tq % P == 0 and tk % P == 0 and h % P == 0

    nc = bacc.Bacc("TRN2", debug=False, target_bir_lowering=False)
    q_d = nc.dram_tensor("query", [b_pc, tq, h], F32, kind="ExternalInput").ap()
    k_d = nc.dram_tensor("memory_bank", [b_pc, tk, h], F32, kind="ExternalInput").ap()
    s_d = nc.dram_tensor("scores", [b_pc, tq, tk], F32, kind="ExternalOutput").ap()
    c_d = nc.dram_tensor("context", [b_pc, tq, h], F32, kind="ExternalOutput").ap()

    with tile.TileContext(nc) as tc:
        with (
            tc.tile_pool(name="singles", bufs=1) as singles,
            tc.tile_pool(name="kn", bufs=1) as kn_pool,
            tc.tile_pool(name="kt", bufs=2) as kt_pool,
            tc.tile_pool(name="knr", bufs=1) as knr_pool,
            tc.tile_pool(name="qraw", bufs=2) as qraw_pool,
            tc.tile_pool(name="qt", bufs=2) as qt_pool,
            tc.tile_pool(name="ev", bufs=2) as e_pool,
            tc.tile_pool(name="sout", bufs=2) as s_pool,
            tc.tile_pool(name="et", bufs=2) as et_pool,
            tc.tile_pool(name="cout", bufs=2) as c_pool,
            tc.tile_pool(name="stats", bufs=6) as stats_pool,
            tc.tile_pool(name="ps_s", bufs=2, space="PSUM") as ps_s_pool,
            tc.tile_pool(name="ps_c", bufs=1, space="PSUM") as ps_c_pool,
            tc.tile_pool(name="ps_t", bufs=2, space="PSUM") as ps_t_pool,
        ):
            ident = singles.tile([P, P], F32)
            make_identity(nc, ident)

            def body(_iv=None):
                # one software-pipelined pass over (batch, q-tile)

                def pre_kn(b):
                    kn = kn_pool.tile([P, nk, h], F32, tag="kn")
                    for j in range(nk):
                        nc.sync.dma_start(
                            out=kn[:, j, :], in_=k_d[b, j * P : (j + 1) * P, :]
                        )
                    return kn

                def pre_kt_alloc():
                    # K^T: kt[p, i, j*P:(j+1)*P] = K[j*P+0.., i*P+p]
                    kt = kt_pool.tile([P, nh, tk], F32R, tag="kt")
                    return kt

                def pre_kt_half(kn, kt, j0):
                    jj = min(4, nk - j0)
                    for i in range(nh):
                        pt = ps_t_pool.tile([P, 4, P], F32, tag="pt")
                        for j in range(j0, j0 + jj):
                            nc.tensor.transpose(
                                pt[:, j - j0, :],
                                kn[:, j, i * P : (i + 1) * P],
                                ident,
                            )
                        if i % 2 == 0:
                            nc.vector.tensor_copy(
                                kt[:, i, j0 * P : (j0 + jj) * P], pt[:, :jj, :]
                            )
                        else:
                            nc.scalar.copy(
                                kt[:, i, j0 * P : (j0 + jj) * P], pt[:, :jj, :]
                            )

                def pre_knr(kn):
                    # K rounded to fp32r for the context matmul
                    knr = knr_pool.tile([P, nk, h], F32R, tag="knr")
                    for j in range(nk):
                        nc.scalar.copy(knr[:, j, :], kn[:, j, :])
                    return knr

                def preamble(b):
                    kn = pre_kn(b)
                    kt = pre_kt_alloc()
                    for j0 in range(0, nk, 4):
                        pre_kt_half(kn, kt, j0)
                    knr = pre_knr(kn)
                    return kt, knr, kn

                def produce_qt(b, qt, kn):
                    if strip_dma:
                        qraw = kn[:, qt % nk, :]
                    else:
                        qraw = qraw_pool.tile([P, h], F32, tag="qraw")
                        nc.sync.dma_start(
                            out=qraw, in_=q_d[b, qt * P : (qt + 1) * P, :]
                        )
                    qtt = qt_pool.tile([P, nh, P], F32R, tag="qt")
                    for g, i0 in enumerate(range(0, nh, 4)):
                        ii = min(4, nh - i0)
                        pt = ps_t_pool.tile([P, 4, P], F32, tag="pt")
                        for i in range(i0, i0 + ii):
                            nc.tensor.transpose(
                                pt[:, i - i0, :], qraw[:, i * P : (i + 1) * P], ident
                            )
                        if g % 2 == 0:
                            nc.scalar.copy(qtt[:, i0 : i0 + ii, :], pt[:, :ii, :])
                        else:
                            nc.vector.tensor_copy(qtt[:, i0 : i0 + ii, :], pt[:, :ii, :])
                    return qtt

                def s_phase(b, qt, qtt, kt):
                    ps_s = ps_s_pool.tile([P, tk], F32, tag="ps_s")
                    negm_parts = []
                    for ci, (off, w) in enumerate(_mm_chunks(tk)):
                        for i in range(nh):
                            nc.tensor.matmul(
                                ps_s[:, off : off + w],
                                qtt[:, i, :],
                                kt[:, i, off : off + w],
                                start=(i == 0),
                                stop=(i == nh - 1),
                            )
                        # per-chunk -max, so the row max is ready right when
                        # the last chunk's matmuls finish
                        nm = stats_pool.tile([P, 1], F32, tag=f"negm{ci}")
                        nc.vector.reduce_max(
                            out=nm,
                            in_=ps_s[:, off : off + w],
                            axis=mybir.AxisListType.X,
                            negate=True,
                        )
                        negm_parts.append(nm)
                    return ps_s, negm_parts

                def softmax_et(b, qt, ps_s, negm_parts):
                    if len(negm_parts) == 1:
                        negm = negm_parts[0]
                    else:
                        negm = stats_pool.tile([P, 1], F32, tag="negm")
                        nc.vector.tensor_tensor(
                            out=negm,
                            in0=negm_parts[0],
                            in1=negm_parts[1],
                            op=mybir.AluOpType.min,
                        )
                        for nm in negm_parts[2:]:
                            nc.vector.tensor_tensor(
                                out=negm, in0=negm, in1=nm, op=mybir.AluOpType.min
                            )
                    ev = e_pool.tile([P, tk], F32, tag="ev")
                    esums = []
                    for ci, (off, w) in enumerate(_mm_chunks(tk)):
                        es = stats_pool.tile([P, 1], F32, tag=f"esum{ci}")
                        nc.vector.memset(es, 0.0)
                        nc.scalar.activation(
                            out=ev[:, off : off + w],
                            in_=ps_s[:, off : off + w],
                            func=mybir.ActivationFunctionType.Exp,
                            bias=negm,
                            scale=1.0,
                            accum_out=es,
                        )
                        esums.append(es)
                    if len(esums) == 1:
                        esum = esums[0]
                    else:
                        esum = stats_pool.tile([P, 1], F32, tag="esum")
                        nc.vector.tensor_add(esum, esums[0], esums[1])
                        for es in esums[2:]:
                            nc.vector.tensor_add(esum, esum, es)
                    # raw scores: PSUM -> SBUF staging -> DRAM
                    sout = s_pool.tile([P, tk], F32, tag="sout")
                    for ci, (off, w) in enumerate(_mm_chunks(tk)):
                        if ci % 2 == 0:
                            nc.vector.tensor_copy(
                                sout[:, off : off + w], ps_s[:, off : off + w]
                            )
                        else:
                            nc.scalar.copy(
                                sout[:, off : off + w], ps_s[:, off : off + w]
                            )
                    if not strip_dma:
                        nc.gpsimd.dma_start(
                            out=s_d[b, qt * P : (qt + 1) * P, :], in_=sout
                        )
                    r = stats_pool.tile([P, 1], F32, tag="r")
                    nc.vector.reciprocal(r, esum)
                    # E^T tiles
                    ett = et_pool.tile([P, nk, P], F32R, tag="et")
                    for g, j0 in enumerate(range(0, nk, 4)):
                        jj = min(4, nk - j0)
                        pt = ps_t_pool.tile([P, 4, P], F32, tag="pt")
                        for j in range(j0, j0 + jj):
                            nc.tensor.transpose(
                                pt[:, j - j0, :], ev[:, j * P : (j + 1) * P], ident
                            )
                        if g % 2 == 0:
                            nc.vector.tensor_copy(ett[:, j0 : j0 + jj, :], pt[:, :jj, :])
                        else:
                            nc.scalar.copy(ett[:, j0 : j0 + jj, :], pt[:, :jj, :])
                    return ett, r

                def c_phase(b, qt, ett, r, knr):
                    ps_c = ps_c_pool.tile([P, h], F32, tag="ps_c")
                    for off, w in _mm_chunks(h):
                        for j in range(nk):
                            nc.tensor.matmul(
                                ps_c[:, off : off + w],
                                ett[:, j, :],
                                knr[:, j, off : off + w],
                                start=(j == 0),
                                stop=(j == nk - 1),
                            )
                    cout = c_pool.tile([P, h], F32, tag="cout")
                    for ci, (off, w) in enumerate(_mm_chunks(h)):
                        if ci % 2 == 0:
                            nc.vector.tensor_scalar_mul(
                                cout[:, off : off + w], ps_c[:, off : off + w], r
                            )
                        else:
                            nc.scalar.mul(cout[:, off : off + w], ps_c[:, off : off + w], r)
                    if not strip_dma or (b == b_pc - 1 and qt == nq - 1):
                        nc.gpsimd.dma_start(
                            out=c_d[b, qt * P : (qt + 1) * P, :], in_=cout
                        )

                # Pipelined emission, grouped so the PE alternates one
                # transpose phase (QTt(t) + ETt(t-1)) with one long matmul
                # phase (C(t-1) + S(t)) to keep the PE clock-gate warm.
                tiles = [(b, qt) for b in range(b_pc) for qt in range(nq)]
                prev = None  # (b, qt, ps_s, knr)
                cur = {}
                pending = {}  # partially-built next-batch state
                for b, qt in tiles:
                    if b == 0 and qt == 0:
                        cur[0] = preamble(0)
                    elif qt == 0:
                        # finish any slices not emitted during the previous batch
                        pk = pending.pop(b, None)
                        if pk is None:
                            cur[b] = preamble(b)
                        else:
                            cur[b] = (pk["kt"], pre_knr(pk["kn"]), pk["kn"])
                    nxt = b + 1
                    if nxt < b_pc and nq >= 8:
                        # spread next batch's K load + K^T build over this batch
                        if qt == nq - 4:
                            pending[nxt] = {"kn": pre_kn(nxt)}
                        elif qt == nq - 3:
                            pending[nxt]["kt"] = pre_kt_alloc()
                            pre_kt_half(pending[nxt]["kn"], pending[nxt]["kt"], 0)
                        elif qt == nq - 2:
                            pre_kt_half(pending[nxt]["kn"], pending[nxt]["kt"], 4)
                    qtt = produce_qt(b, qt, cur[b][2])
                    if prev is not None:
                        pb, pqt, ps_s_prev, pnm, pknr = prev
                        ett, r = softmax_et(pb, pqt, ps_s_prev, pnm)
                        c_phase(pb, pqt, ett, r, pknr)
                    ps_s, negm_parts = s_phase(b, qt, qtt, cur[b][0])
                    prev = (b, qt, ps_s, negm_parts, cur[b][1])
                pb, pqt, ps_s_prev, pnm, pknr = prev
                ett, r = softmax_et(pb, pqt, ps_s_prev, pnm)
                c_phase(pb, pqt, ett, r, pknr)

            if repeats == 1:
                body()
            else:
                with tc.For_i(
                    0, repeats, 1, hint_engines=(mybir.EngineType.PE,)
                ) as iv:
                    body(iv)

    nc.compile()
    return nc


_NC_CACHE = {}


def _get_nc(repeats=1):
    key = repeats
    if key not in _NC_CACHE:
        _NC_CACHE[key] = build_attention_nc(repeats=repeats)
    return _NC_CACHE[key]


def run_on_hw(query, memory_bank, repeats=1):
    nc = _get_nc(repeats)
    query = np.ascontiguousarray(query, dtype=np.float32)
    memory_bank = np.ascontiguousarray(memory_bank, dtype=np.float32)
    in_maps = [
        {
            "query": query[c * B_PC : (c + 1) * B_PC],
            "memory_bank": memory_bank[c * B_PC : (c + 1) * B_PC],
        }
        for c in range(N_CORES)
    ]
    res = run_bass_kernel_spmd(nc, in_maps, core_ids=list(range(N_CORES)))
    context = np.concatenate([res.results[c]["context"] for c in range(N_CORES)], axis=0)
    scores = np.concatenate([res.results[c]["scores"] for c in range(N_CORES)], axis=0)
    return context, scores


def kernel(query, memory_bank):
    return run_on_hw(query, memory_bank, repeats=1)



# revision 2
# speedup vs baseline: 1.2145x; 1.2145x over previous
"""Trainium2 Bass kernel for nn_AttentionLayer (B=16, TQ=TK=H=1024, fp32).

reference:
    scores  = einsum('bqh,bkh->bqk', query, memory_bank)
    probs   = softmax(scores, axis=2)
    context = einsum('bqk,bkh->bqh', probs, memory_bank)
    return (context, scores)

Sharding: batch dim split across 8 NeuronCores (2 batches per core), no
cross-device communication.

v2 design notes (vs baseline):
  - K is cast-DMA'd (gpsimd) straight into an fp32r SBUF tile: no f32
    staging copy; the PE's TF32 rounding happens on operand read.
  - All PE transposes use an fp32r identity as the moving operand
    (1.5 cyc/row instead of fp32's 2; 16-bit identities are rejected by
    the BIR verifier when the data is 32-bit).
  - Steady-state PE stream per q-tile t:
        ET(t-1) | S1(t) | QT(t+1) | S2(t) | C(t-1)
    Every PE instruction's dependencies (engine staging copies, exp,
    PSUM WARs) resolve at least one long matmul phase earlier, so the PE
    never stalls and its DVFS p-state stays at max.
  - PSUM: ps_s 2x2 banks, ps_c 1x2 banks, ps_t 2x1 banks = 8 banks.
"""

import numpy as np

import concourse.bass as bass
import concourse.mybir as mybir
import concourse.tile as tile
from concourse import bacc
from concourse.masks import make_identity
from concourse.bass_utils import run_bass_kernel_spmd

N_CORES = 8
B, TQ, TK, H = 16, 1024, 1024, 1024
B_PC = B // N_CORES
P = 128

F32 = mybir.dt.float32
F32R = mybir.dt.float32r
BF16 = mybir.dt.bfloat16

CH = 512  # psum-bank-sized matmul chunk


def build_attention_nc(b_pc=B_PC, tq=TQ, tk=TK, h=H, repeats=1):
    nq, nk, nh = tq // P, tk // P, h // P
    assert tq % P == 0 and tk % P == 0 and h % P == 0
    n_tiles = b_pc * nq
    n_ch = tk // CH  # chunks over the key dim (2)

    nc = bacc.Bacc("TRN2", debug=False, target_bir_lowering=False)
    q_d = nc.dram_tensor("query", [b_pc, tq, h], F32, kind="ExternalInput").ap()
    k_d = nc.dram_tensor("memory_bank", [b_pc, tk, h], F32, kind="ExternalInput").ap()
    s_d = nc.dram_tensor("scores", [b_pc, tq, tk], F32, kind="ExternalOutput").ap()
    c_d = nc.dram_tensor("context", [b_pc, tq, h], F32, kind="ExternalOutput").ap()

    with tile.TileContext(nc) as tc:
        with (
            tc.tile_pool(name="singles", bufs=1) as singles,
            tc.tile_pool(name="knr", bufs=2) as knr_pool,
            tc.tile_pool(name="kt", bufs=2) as kt_pool,
            tc.tile_pool(name="qraw", bufs=3) as qraw_pool,
            tc.tile_pool(name="qt", bufs=2) as qt_pool,
            tc.tile_pool(name="ev", bufs=2) as ev_pool,
            tc.tile_pool(name="et", bufs=2) as et_pool,
            tc.tile_pool(name="sout", bufs=2) as s_pool,
            tc.tile_pool(name="cout", bufs=2) as c_pool,
            tc.tile_pool(name="st", bufs=12) as st_pool,
            tc.tile_pool(name="rr", bufs=3) as r_pool,
            tc.tile_pool(name="ps_s", bufs=2, space="PSUM") as ps_s_pool,
            tc.tile_pool(name="ps_c", bufs=1, space="PSUM") as ps_c_pool,
            tc.tile_pool(name="ps_t", bufs=2, space="PSUM") as ps_t_pool,
        ):
            ident_f32 = singles.tile([P, P], F32)
            make_identity(nc, ident_f32)
            ident = singles.tile([P, P], F32R)
            nc.vector.tensor_copy(ident, ident_f32)

            def body(_iv=None):
                # ---- per-iteration state handles ----
                knr = [None] * b_pc  # [P, nk, h] F32R, DMA'd from DRAM
                kt = [None] * b_pc  # [P, nh, tk] F32R (K^T)
                qraw = [None] * n_tiles  # [P, h] F32R
                qtt = [None] * n_tiles  # [P, nh, P] F32R (Q^T)
                ev = [None] * n_tiles  # [P, tk] F32R (exp(S - max))
                ett = [None] * n_tiles  # [P, nk, P] F32R (E^T)
                ps_s = [None] * n_tiles
                ps_c = [None] * n_tiles
                souts = [None] * n_tiles
                couts = [None] * n_tiles
                negm = [None] * n_tiles
                esums = [None] * n_tiles
                rs = [None] * n_tiles

                def bat(u):
                    return u // nq

                def qof(u):
                    return u % nq

                def dma_knr(b, j0, jn):
                    if knr[b] is None:
                        knr[b] = knr_pool.tile([P, nk, h], F32R, name="knr", tag="knr")
                    for j in range(j0, j0 + jn):
                        nc.gpsimd.dma_start(
                            out=knr[b][:, j, :], in_=k_d[b, j * P : (j + 1) * P, :]
                        )

                def dma_qraw(u):
                    if u >= n_tiles:
                        return
                    b, qt_i = bat(u), qof(u)
                    qraw[u] = qraw_pool.tile([P, h], F32R, name="qraw", tag="qraw")
                    nc.gpsimd.dma_start(
                        out=qraw[u], in_=q_d[b, qt_i * P : (qt_i + 1) * P, :]
                    )

                def kt_group(b, i, jh, eng):
                    """One K^T transpose group: kt[:, i, jh*4*P:(jh*4+4)*P]
                    (4 of the nk*nh 128x128 tiles), reading knr slices
                    j=4*jh..4*jh+3 at column block i."""
                    if kt[b] is None:
                        kt[b] = kt_pool.tile([P, nh, tk], F32R, name="kt", tag="kt")
                    j0 = jh * 4
                    pt = ps_t_pool.tile([P, 4, P], F32R, name="pt", tag="pt")
                    for j in range(j0, j0 + 4):
                        nc.tensor.transpose(
                            pt[:, j - j0, :],
                            knr[b][:, j, i * P : (i + 1) * P],
                            ident,
                        )
                    dst = kt[b][:, i, j0 * P : (j0 + 4) * P]
                    if eng == 0:
                        nc.vector.tensor_copy(dst, pt)
                    else:
                        nc.scalar.copy(dst, pt)

                def et_phase(u):
                    """ET(u): transpose ev(u) -> ett(u); copies chase."""
                    ett[u] = et_pool.tile([P, nk, P], F32R, name="ett", tag="et")
                    for half in range(2):
                        j0 = half * 4
                        pt = ps_t_pool.tile([P, 4, P], F32R, name="pt", tag="pt")
                        for j in range(j0, j0 + 4):
                            nc.tensor.transpose(
                                pt[:, j - j0, :],
                                ev[u][:, j * P : (j + 1) * P],
                                ident,
                            )
                        dst = ett[u][:, j0 : j0 + 4, :]
                        if half == 0:
                            nc.vector.tensor_copy(dst, pt)
                        else:
                            nc.scalar.copy(dst, pt)

                def qt_phase(u):
                    """QT(u): transpose qraw(u) -> qtt(u); copies chase."""
                    if u >= n_tiles:
                        return
                    qtt[u] = qt_pool.tile([P, nh, P], F32R, name="qtt", tag="qt")
                    for half in range(2):
                        i0 = half * 4
                        pt = ps_t_pool.tile([P, 4, P], F32R, name="pt", tag="pt")
                        for i in range(i0, i0 + 4):
                            nc.tensor.transpose(
                                pt[:, i - i0, :],
                                qraw[u][:, i * P : (i + 1) * P],
                                ident,
                            )
                        dst = qtt[u][:, i0 : i0 + 4, :]
                        if half == 0:
                            nc.vector.tensor_copy(dst, pt)
                        else:
                            nc.scalar.copy(dst, pt)

                def s_chunk(u, ci):
                    """S matmul accumulation group for chunk ci of tile u."""
                    b = bat(u)
                    if ps_s[u] is None:
                        ps_s[u] = ps_s_pool.tile([P, tk], F32, name="ps_s", tag="ps_s")
                    off = ci * CH
                    for i in range(nh):
                        nc.tensor.matmul(
                            ps_s[u][:, off : off + CH],
                            qtt[u][:, i, :],
                            kt[b][:, i, off : off + CH],
                            start=(i == 0),
                            stop=(i == nh - 1),
                        )

                def negmax_chunk(u, ci):
                    nm = st_pool.tile([P, 1], F32, name=f"nm{ci}", tag=f"nm{ci}")
                    off = ci * CH
                    nc.vector.reduce_max(
                        out=nm,
                        in_=ps_s[u][:, off : off + CH],
                        axis=mybir.AxisListType.X,
                        negate=True,
                    )
                    return nm

                def c_phase(u, mid=None):
                    """C(u): 2*nk matmuls ett(u) x knr -> ps_c. `mid` is an
                    optional callback emitted between the two chunks (used to
                    slot next-batch kt transposes where their ps_t WAR and
                    staging copies are already settled)."""
                    b = bat(u)
                    ps_c[u] = ps_c_pool.tile([P, h], F32, name="ps_c", tag="ps_c")
                    for ci in range(n_ch):
                        off = ci * CH
                        for j in range(nk):
                            nc.tensor.matmul(
                                ps_c[u][:, off : off + CH],
                                ett[u][:, j, :],
                                knr[b][:, j, off : off + CH],
                                start=(j == 0),
                                stop=(j == nk - 1),
                            )
                        if ci == 0 and mid is not None:
                            mid()

                def sout_half(u, ci):
                    if souts[u] is None:
                        souts[u] = s_pool.tile([P, tk], F32, name="sout", tag="sout")
                    off = ci * CH
                    src = ps_s[u][:, off : off + CH]
                    dst = souts[u][:, off : off + CH]
                    nc.scalar.copy(dst, src)

                def sout_dma(u):
                    b, qt_i = bat(u), qof(u)
                    nc.gpsimd.dma_start(
                        out=s_d[b, qt_i * P : (qt_i + 1) * P, :], in_=souts[u]
                    )

                def cout_half(u, ci, eng):
                    if couts[u] is None:
                        couts[u] = c_pool.tile([P, h], F32, name="cout", tag="cout")
                    off = ci * CH
                    src = ps_c[u][:, off : off + CH]
                    dst = couts[u][:, off : off + CH]
                    if eng == 0:
                        nc.vector.tensor_scalar_mul(dst, src, rs[u])
                    else:
                        nc.scalar.mul(dst, src, rs[u])

                def cout_dma(u):
                    b, qt_i = bat(u), qof(u)
                    nc.gpsimd.dma_start(
                        out=c_d[b, qt_i * P : (qt_i + 1) * P, :], in_=couts[u]
                    )

                def exp_phase(u):
                    """negm combine + exp both chunks (ACT) + esum/recip."""
                    nm0, nm1 = negm[u]
                    nmc = st_pool.tile([P, 1], F32, name="nmc", tag="nmc")
                    nc.vector.tensor_tensor(
                        out=nmc, in0=nm0, in1=nm1, op=mybir.AluOpType.min
                    )
                    ev[u] = ev_pool.tile([P, tk], F32R, name="ev", tag="ev")
                    ess = []
                    for ci in range(n_ch):
                        es = st_pool.tile([P, 1], F32, name=f"es{ci}", tag=f"es{ci}")
                        nc.vector.memset(es, 0.0)
                        off = ci * CH
                        nc.scalar.activation(
                            out=ev[u][:, off : off + CH],
                            in_=ps_s[u][:, off : off + CH],
                            func=mybir.ActivationFunctionType.Exp,
                            bias=nmc,
                            scale=1.0,
                            accum_out=es,
                        )
                        ess.append(es)
                    esum = st_pool.tile([P, 1], F32, name="esum", tag="esum")
                    nc.vector.tensor_add(esum, ess[0], ess[1])
                    rs[u] = r_pool.tile([P, 1], F32, name="r", tag="r")
                    nc.vector.reciprocal(rs[u], esum)

                # ================= prologue =================
                # K(0) in, then K^T(0) build (jh=0 groups first so the
                # transposes only ever wait on already-arrived K slices),
                # then Q^T(0).
                dma_knr(0, 0, nk)
                dma_qraw(0)
                dma_qraw(1)
                for jh in range(2):
                    for i in range(nh):
                        kt_group(0, i, jh, i % 2)
                qt_phase(0)

                # ================= main loop =================
                # iteration u: ET(u-1) | S1(u) | QT(u+1) | S2(u) | C(u-1)
                # with next-batch K prep spread over the current batch:
                #   qof 0..3: 2 knr slice DMAs per iteration
                #   qof 4..7: 4 kt transpose groups per iteration (2 after
                #             S2, 2 between the C chunks)
                for u in range(n_tiles + 2):
                    t, tp, tpp, tn = u, u - 1, u - 2, u + 1
                    has_t = t < n_tiles
                    has_tp = 0 <= tp < n_tiles
                    has_tpp = 0 <= tpp < n_tiles

                    prep_dma = prep_kt = None
                    if has_t and bat(t) + 1 < b_pc:
                        nb = bat(t) + 1
                        if qof(t) <= 3:
                            prep_dma = (nb, qof(t) * 2)
                        else:
                            prep_kt = (nb, qof(t) - 4)  # m in 0..3

                    if prep_dma is not None:
                        dma_knr(prep_dma[0], prep_dma[1], 2)

                    # --- ET(u-1) ---
                    if has_tp:
                        et_phase(tp)
                    # --- S1(u) ---
                    if has_t:
                        s_chunk(t, 0)
                        nm0 = negmax_chunk(t, 0)
                    # sout/cout staging for earlier tiles (run during S1/S2)
                    if has_tp:
                        sout_half(tp, 0)
                    if has_tpp:
                        cout_half(tpp, 0, 0)
                        cout_half(tpp, 1, 1)
                        cout_dma(tpp)
                    # --- QT(u+1) ---
                    if tn < n_tiles:
                        qt_phase(tn)
                    # --- S2(u) ---
                    if has_t:
                        s_chunk(t, 1)
                        nm1 = negmax_chunk(t, 1)
                        negm[t] = (nm0, nm1)
                    if has_tp:
                        sout_half(tp, 1)
                        sout_dma(tp)
                    # --- next-batch kt groups: 2 post-S2 (their ps_t WARs
                    # cleared during S2), 2 mid-C below ---
                    def kt_pair(which):
                        nb, m = prep_kt
                        i = 2 * m + which
                        kt_group(nb, i, 0, 0)
                        kt_group(nb, i, 1, 1)

                    if prep_kt is not None:
                        kt_pair(0)
                    # --- C(u-1) ---
                    if has_tp:
                        c_phase(
                            tp,
                            mid=(lambda: kt_pair(1)) if prep_kt is not None else None,
                        )
                    elif prep_kt is not None:
                        kt_pair(1)
                    # --- softmax of tile u (runs during C window) ---
                    if has_t:
                        exp_phase(t)
                    # prefetch q two tiles ahead
                    if tn + 1 < n_tiles:
                        dma_qraw(tn + 1)

            if repeats == 1:
                body()
            else:
                with tc.For_i(
                    0, repeats, 1, hint_engines=(mybir.EngineType.PE,)
                ) as iv:
                    body(iv)

    nc.compile()
    return nc


_NC_CACHE = {}


def _get_nc(repeats=1):
    key = repeats
    if key not in _NC_CACHE:
        _NC_CACHE[key] = build_attention_nc(repeats=repeats)
    return _NC_CACHE[key]


def run_on_hw(query, memory_bank, repeats=1):
    nc = _get_nc(repeats)
    query = np.ascontiguousarray(query, dtype=np.float32)
    memory_bank = np.ascontiguousarray(memory_bank, dtype=np.float32)
    in_maps = [
        {
            "query": query[c * B_PC : (c + 1) * B_PC],
            "memory_bank": memory_bank[c * B_PC : (c + 1) * B_PC],
        }
        for c in range(N_CORES)
    ]
    res = run_bass_kernel_spmd(nc, in_maps, core_ids=list(range(N_CORES)))
    context = np.concatenate(
        [res.results[c]["context"] for c in range(N_CORES)], axis=0
    )
    scores = np.concatenate(
        [res.results[c]["scores"] for c in range(N_CORES)], axis=0
    )
    return context, scores


def kernel(query, memory_bank):
    return run_on_hw(query, memory_bank, repeats=1)
